# revision 2
# baseline (speedup 1.0000x reference)
"""CheckInEmbedding kernel V2.

Changes vs staged baseline:
  * Contiguous-descriptor loads: poi rows land in a compact in-tile
    [128, r*128] (one r*512 B chunk per partition -> 128 descriptors,
    line-rate) instead of 512 B descriptors into interleaved slots.
  * DVE leaky_relu reads the compact tile, writes the interleaved output
    slots (strided writes are free on DVE).
  * Per-buffer user-column prefill is emitted just before the buffer's
    first use, so the first stores don't wait on ALL prefills.
  * Small first tile (fast pipeline fill) and small last tile (short
    non-overlapped store drain).
"""

import numpy as np

N = 500000
DPOI = 128
DU = 128
DOUT = DPOI + DU
NCORES = 8
ROWS_PER_CORE = N // NCORES  # 62500
GROUPS = 489  # ceil(62500 / 128)
PAD_ROWS = GROUPS * 128  # 62592
# sum == GROUPS; small head tile fills the pipe fast, small tail tile
# keeps the final non-overlappable store drain short.
TILE_SCHEDULE = [9] + [40] * 11 + [24] + [16]
NBUFS = 3  # output tiles (rmax*1024 B per partition each)
NIBUFS = 3  # input tiles (rmax*512 B per partition each)

_prog_cache = {}


def _prefill(nc, t, usr, rmax):
    v = t[:].rearrange("p (q c) -> p q c", c=DOUT)
    nc.vector.tensor_copy(
        out=v[:, 0:1, DPOI:DOUT],
        in_=usr[:].rearrange("p (q c) -> p q c", q=1),
    )
    q = 1
    while q < rmax:
        step = min(q, rmax - q)
        nc.vector.tensor_copy(
            out=v[:, q : q + step, DPOI:DOUT],
            in_=v[:, 0:step, DPOI:DOUT],
        )
        q += step


def _emit_pass(nc, mybir, itiles, otiles, poi, out, tile_schedule, usr, rmax, first):
    nobufs = len(otiles)
    nibufs = len(itiles)
    row0 = 0
    for i, r in enumerate(tile_schedule):
        it = itiles[i % nibufs]
        ot = otiles[i % nobufs]
        if first and i < nobufs:
            _prefill(nc, ot, usr, rmax)
        v = ot[:].rearrange("p (q c) -> p q c", c=DOUT)
        rows = r * 128
        src = poi[row0 : row0 + rows, :].rearrange("(p q) d -> p (q d)", q=r)
        nc.sync.dma_start(out=it[:, 0 : r * DPOI], in_=src)
        iv = it[:].rearrange("p (q c) -> p q c", c=DPOI)
        # leaky_relu(x) = max(0.2*x, x)
        nc.vector.scalar_tensor_tensor(
            out=v[:, 0:r, 0:DPOI],
            in0=iv[:, 0:r, :],
            scalar=0.2,
            in1=iv[:, 0:r, :],
            op0=mybir.AluOpType.mult,
            op1=mybir.AluOpType.max,
        )
        dst = out[row0 : row0 + rows, :].rearrange("(p q) c -> p (q c)", q=r)
        nc.scalar.dma_start(out=dst, in_=ot[:, 0 : r * DOUT])
        row0 += rows


def _build_program(pad_rows, tile_schedule, nbufs, repeats=1):
    import concourse.bacc as bacc
    import concourse.mybir as mybir
    from concourse.tile import TileContext

    f32 = mybir.dt.float32
    nc = bacc.Bacc()
    poi = nc.declare_dram_parameter("poi", [pad_rows, DPOI], f32, isOutput=False)
    ublk = nc.declare_dram_parameter("ublk", [128, DU], f32, isOutput=False)
    out = nc.declare_dram_parameter("out", [pad_rows, DOUT], f32, isOutput=True)

    rmax = max(tile_schedule)
    with TileContext(nc) as tc:
        with (
            tc.tile_pool(name="obuf", bufs=1) as pool,
            tc.tile_pool(name="ibuf", bufs=1) as ipool,
            tc.tile_pool(name="ubuf", bufs=1) as upool,
        ):
            usr = upool.tile([128, DU], f32)
            nc.sync.dma_start(out=usr[:], in_=ublk[:])

            otiles = [
                pool.tile([128, rmax * DOUT], f32, name=f"obuf{b}")
                for b in range(nbufs)
            ]
            itiles = [
                ipool.tile([128, rmax * DPOI], f32, name=f"ibuf{b}")
                for b in range(NIBUFS)
            ]
            for rep in range(repeats):
                _emit_pass(
                    nc, mybir, itiles, otiles, poi, out, tile_schedule,
                    usr, rmax, first=(rep == 0),
                )
    nc.compile()
    return nc


def _get_program(pad_rows, tile_schedule, nbufs, repeats=1):
    key = (pad_rows, tuple(tile_schedule), nbufs, repeats)
    if key not in _prog_cache:
        _prog_cache[key] = _build_program(pad_rows, tile_schedule, nbufs, repeats)
    return _prog_cache[key]


def _prepare(hot, reg, user, rows_per_core, pad_rows, tile_schedule, nbufs, repeats=1):
    nc = _get_program(pad_rows, tile_schedule, nbufs, repeats)
    poi_full = np.concatenate(
        [np.ascontiguousarray(hot), np.ascontiguousarray(reg)], axis=1
    ).astype(np.float32, copy=False)
    ublk = np.broadcast_to(
        np.asarray(user, dtype=np.float32).reshape(1, DU), (128, DU)
    ).copy()
    in_maps = []
    for c in range(NCORES):
        sl = poi_full[c * rows_per_core : (c + 1) * rows_per_core]
        if pad_rows != rows_per_core:
            p = np.zeros((pad_rows, DPOI), np.float32)
            p[:rows_per_core] = sl
        else:
            p = np.ascontiguousarray(sl)
        in_maps.append({"poi": p, "ublk": ublk})
    return nc, in_maps


def _run(hot, reg, user, rows_per_core, pad_rows, tile_schedule, nbufs, **spmd_kwargs):
    from concourse.bass_utils import run_bass_kernel_spmd

    nc, in_maps = _prepare(
        hot, reg, user, rows_per_core, pad_rows, tile_schedule, nbufs
    )
    res = run_bass_kernel_spmd(nc, in_maps, list(range(NCORES)), **spmd_kwargs)
    outs = [res.results[c]["out"][:rows_per_core] for c in range(NCORES)]
    return np.concatenate(outs, axis=0), res


def kernel(hotness_embedding_list, region_embedding_list, user_embedding):
    out, _ = _run(
        hotness_embedding_list,
        region_embedding_list,
        user_embedding,
        ROWS_PER_CORE,
        PAD_ROWS,
        TILE_SCHEDULE,
        NBUFS,
    )
    return out
